# revision 8
# baseline (speedup 1.0000x reference)
"""AttnBlock (GroupNorm + cross-attention + proj + residual) on 8 trn2 cores.

Problem (hardcoded shapes): x, c: [2, 128, 16, 16, 16] fp32; C=128 channels,
N=4096 spatial tokens, 4 groups of 32 channels.

  h  = GN(x; g1, b1)            c_ = GN(c; g2, b2)
  q = wq c_ + bq ; k = wk h + bk ; v = wv h + bv
  S[b,i,j] = <q[:,i], k[:,j]> / sqrt(C) ;  A = softmax_j(S)
  out = x + wp (v A^T) + bp

Sharding: 8 cores, core m -> batch b=m//4, query rows i0=(m%4)*1024 .. +1024.
Each core recomputes GN + K/V^T for its batch (cheap), computes its
[1024 x 4096] slice of exp(S^T) with j on partitions (transpose-free layout),
accumulates V^T @ P and the softmax denominator in PSUM, normalizes, projects,
adds residual, and returns its [128, 1024] output slice.

Matmul dtype: float32r (full-rate on the PE; fp32 runs at 1/4 rate).
Stats / broadcast matmuls stay exact fp32.
"""

import numpy as np

import concourse.bass as bass
import concourse.tile as tile
from concourse import mybir
from concourse.bass_utils import run_bass_kernel_spmd

N_CORES = 8
C = 128
N = 4096          # tokens per batch
I = 1024          # query rows per core
NG = 4            # groups
EPS = 1e-6
SCALE = 1.0 / np.sqrt(C)
JB = N // 128     # 32 j-blocks
F32 = mybir.dt.float32
F32R = mybir.dt.float32r

MM_DT = F32R      # dtype knob for the big matmuls

DEBUG_OUTS = True


def cap_sync_waits(nc):
    """Split multi-wait instructions: the pinned walrus accepts at most one
    sync wait per instruction ("Too many sync wait commands"). Hoist extra
    waits into single-wait NOPs inserted just before, on the same engine."""
    ctr = 0
    for f in nc.m.functions:
        for b in f.blocks:
            out = []
            for inst in b.instructions:
                si = inst.sync_info
                if si is not None and si.on_wait and len(si.on_wait) > 1:
                    waits = list(si.on_wait)
                    for w in waits[:-1]:
                        ctr += 1
                        out.append(mybir.InstNoOp(
                            name=f"I-waitsplit-{ctr}",
                            engine=inst.engine,
                            bass_nofuse=True,
                            sync_info=mybir.SyncInfo(on_wait=[w], on_update=[]),
                        ))
                    si.on_wait = waits[-1:]
                out.append(inst)
            b.instructions = out


def _r(ap):
    """View an fp32 AP as float32r for full-rate PE matmuls."""
    if MM_DT is F32:
        return ap
    return ap.bitcast(MM_DT)


def build_program():
    nc = bass.Bass("TRN2", target_bir_lowering=False, debug=False)

    # I/O
    xb = nc.declare_dram_parameter("xb", [C, N], F32, isOutput=False)
    cb = nc.declare_dram_parameter("cb", [C, N], F32, isOutput=False)
    x_sl = nc.declare_dram_parameter("x_sl", [C, I], F32, isOutput=False)
    c_sl = nc.declare_dram_parameter("c_sl", [C, I], F32, isOutput=False)
    wqT = nc.declare_dram_parameter("wqT", [C, C], F32, isOutput=False)
    wkT = nc.declare_dram_parameter("wkT", [C, C], F32, isOutput=False)
    wvT = nc.declare_dram_parameter("wvT", [C, C], F32, isOutput=False)
    wpT = nc.declare_dram_parameter("wpT", [C, C], F32, isOutput=False)
    bq_d = nc.declare_dram_parameter("bq", [C, 1], F32, isOutput=False)
    bk_d = nc.declare_dram_parameter("bk", [C, 1], F32, isOutput=False)
    btp_d = nc.declare_dram_parameter("btp", [C, 1], F32, isOutput=False)
    g1_d = nc.declare_dram_parameter("g1", [C, 1], F32, isOutput=False)
    b1_d = nc.declare_dram_parameter("b1", [C, 1], F32, isOutput=False)
    g2_d = nc.declare_dram_parameter("g2", [C, 1], F32, isOutput=False)
    b2_d = nc.declare_dram_parameter("b2", [C, 1], F32, isOutput=False)
    gavg_d = nc.declare_dram_parameter("gavg", [C, NG], F32, isOutput=False)
    bc4_d = nc.declare_dram_parameter("bc4", [NG, C], F32, isOutput=False)
    ones_d = nc.declare_dram_parameter("ones", [C, 1], F32, isOutput=False)
    ones1_d = nc.declare_dram_parameter("ones1", [1, C], F32, isOutput=False)
    y = nc.declare_dram_parameter("y", [C, I], F32, isOutput=True)

    dbg = {}
    if DEBUG_OUTS:
        for nm, shp in [
            ("dbg_h", [C, N]), ("dbg_k", [C, N]), ("dbg_q", [C, I]),
            ("dbg_vt", [C, N]), ("dbg_d", [1, I]), ("dbg_o", [C, I]),
        ]:
            dbg[nm] = nc.declare_dram_parameter(nm, shp, F32, isOutput=True)

    with tile.TileContext(nc) as tc:
        with (
            tc.tile_pool(name="persist", bufs=1) as per,
            tc.tile_pool(name="smalls", bufs=1) as sm,
            tc.tile_pool(name="ptiles", bufs=3) as pp,
        ):
            # ---- constant / input loads ----
            x_t = per.tile([C, N], F32, tag="x")
            nc.sync.dma_start(x_t[:], xb[:])
            c_t = per.tile([C, N], F32, tag="c")
            nc.sync.dma_start(c_t[:], cb[:])
            xsl_t = per.tile([C, I], F32, tag="xsl")
            nc.sync.dma_start(xsl_t[:], x_sl[:])
            csl_t = per.tile([C, I], F32, tag="csl")
            nc.sync.dma_start(csl_t[:], c_sl[:])

            wq_t = per.tile([C, C], F32, tag="wq")
            nc.sync.dma_start(wq_t[:], wqT[:])
            wk_t = per.tile([C, C], F32, tag="wk")
            nc.sync.dma_start(wk_t[:], wkT[:])
            wv_t = per.tile([C, C], F32, tag="wv")
            nc.sync.dma_start(wv_t[:], wvT[:])
            wp_t = per.tile([C, C], F32, tag="wp")
            nc.sync.dma_start(wp_t[:], wpT[:])

            def load_small(name, dram, shape):
                t = sm.tile(shape, F32, tag=name)
                nc.sync.dma_start(t[:], dram[:])
                return t

            bq_t = load_small("bq", bq_d, [C, 1])
            bk_t = load_small("bk", bk_d, [C, 1])
            btp_t = load_small("btp", btp_d, [C, 1])
            g1_t = load_small("g1", g1_d, [C, 1])
            b1_t = load_small("b1", b1_d, [C, 1])
            g2_t = load_small("g2", g2_d, [C, 1])
            b2_t = load_small("b2", b2_d, [C, 1])
            gavg_t = load_small("gavg", gavg_d, [C, NG])
            bc4_t = load_small("bc4", bc4_d, [NG, C])
            ones_t = load_small("ones", ones_d, [C, 1])
            ones1_t = load_small("ones1", ones1_d, [1, C])

            # round DMA-loaded fp32 operands to float32r for the PE
            wq_r = per.tile([C, C], MM_DT, tag="wq_r")
            nc.vector.tensor_copy(wq_r[:], wq_t[:])
            wk_r = per.tile([C, C], MM_DT, tag="wk_r")
            nc.vector.tensor_copy(wk_r[:], wk_t[:])
            wv_r = per.tile([C, C], MM_DT, tag="wv_r")
            nc.vector.tensor_copy(wv_r[:], wv_t[:])
            wp_r = per.tile([C, C], MM_DT, tag="wp_r")
            nc.vector.tensor_copy(wp_r[:], wp_t[:])
            ones_r = sm.tile([C, 1], MM_DT, tag="ones_r")
            nc.vector.tensor_copy(ones_r[:], ones_t[:])

            eps128_t = sm.tile([C, 1], F32, tag="eps128")
            nc.vector.memset(eps128_t[:], EPS)
            zero128_t = sm.tile([C, 1], F32, tag="zero128")
            nc.vector.memset(zero128_t[:], 0.0)
            zero1_t = sm.tile([1, 1], F32, tag="zero1")
            nc.vector.memset(zero1_t[:], 0.0)

            # ---- group-norm scale/offset per channel: A[c], B[c] ----
            # Per-channel mean/var over 4096 free elems (bn_stats), then
            # aggregate 32-channel groups across partitions via tiny matmuls.
            def gn_affine(src_t, gamma_t, beta_t, label):
                stats = sm.tile([C, 8, 6], F32, tag=f"st_{label}")
                for ch in range(8):
                    nc.vector.bn_stats(
                        out=stats[:, ch, :], in_=src_t[:, ch * 512:(ch + 1) * 512]
                    )
                mv = sm.tile([C, 2], F32, tag=f"mv_{label}")
                nc.vector.bn_aggr(out=mv[:], in_=stats[:])
                # data2 = [mean, mean^2 + var]
                d2 = sm.tile([C, 2], F32, tag=f"d2_{label}")
                nc.vector.tensor_copy(d2[:, 0:1], mv[:, 0:1])
                nc.vector.tensor_mul(d2[:, 1:2], mv[:, 0:1], mv[:, 0:1])
                nc.vector.tensor_add(d2[:, 1:2], d2[:, 1:2], mv[:, 1:2])
                with tc.tile_pool(
                    name=f"gnps_{label}", bufs=1, space=bass.MemorySpace.PSUM
                ) as gnps:
                    gps = gnps.tile([NG, 2], F32, tag="g")
                    nc.tensor.matmul(gps[:], gavg_t[:], d2[:], start=True, stop=True)
                    gsb = sm.tile([NG, 2], F32, tag=f"gsb_{label}")
                    nc.vector.tensor_copy(gsb[:], gps[:])
                    cps = gnps.tile([C, 2], F32, tag="ch")
                    nc.tensor.matmul(cps[:], bc4_t[:], gsb[:], start=True, stop=True)
                    csb = sm.tile([C, 2], F32, tag=f"csb_{label}")
                    nc.vector.tensor_copy(csb[:], cps[:])
                # var = E[x^2] - mu^2 ; rstd = exp(-0.5*ln(var+eps))
                var = sm.tile([C, 1], F32, tag=f"var_{label}")
                nc.vector.tensor_mul(var[:], csb[:, 0:1], csb[:, 0:1])
                nc.vector.tensor_sub(var[:], csb[:, 1:2], var[:])
                lnv = sm.tile([C, 1], F32, tag=f"lnv_{label}")
                nc.scalar.activation(
                    out=lnv[:], in_=var[:], func=mybir.ActivationFunctionType.Ln,
                    bias=eps128_t[:], scale=1.0,
                )
                rstd = sm.tile([C, 1], F32, tag=f"rstd_{label}")
                nc.scalar.activation(
                    out=rstd[:], in_=lnv[:], func=mybir.ActivationFunctionType.Exp,
                    bias=zero128_t[:], scale=-0.5,
                )
                a_t = sm.tile([C, 1], F32, tag=f"A_{label}")
                nc.vector.tensor_mul(a_t[:], rstd[:], gamma_t[:])
                b_t = sm.tile([C, 1], F32, tag=f"B_{label}")
                nc.vector.tensor_mul(b_t[:], csb[:, 0:1], a_t[:])
                nc.vector.tensor_sub(b_t[:], beta_t[:], b_t[:])
                return a_t, b_t

            ax_t, bx_t = gn_affine(x_t, g1_t, b1_t, "x")
            ac_t, bc_t = gn_affine(c_t, g2_t, b2_t, "c")

            # ---- normalize: h = x*A + B (full), cn = c_sl*Ac + Bc (slice) ----
            h_t = per.tile([C, N], MM_DT, tag="h")
            nc.vector.tensor_scalar(
                out=h_t[:], in0=x_t[:], scalar1=ax_t[:], scalar2=bx_t[:],
                op0=mybir.AluOpType.mult, op1=mybir.AluOpType.add,
            )
            cn_t = per.tile([C, I], MM_DT, tag="cn")
            nc.vector.tensor_scalar(
                out=cn_t[:], in0=csl_t[:], scalar1=ac_t[:], scalar2=bc_t[:],
                op0=mybir.AluOpType.mult, op1=mybir.AluOpType.add,
            )
            if DEBUG_OUTS:
                nc.sync.dma_start(dbg["dbg_h"][:], h_t[:].bitcast(F32))

            # ---- projections ----
            k_t = per.tile([C, N], MM_DT, tag="k")
            q_t = per.tile([C, I], MM_DT, tag="q")
            vt_t = per.tile([C, JB, C], MM_DT, tag="vt")   # [j-in-block, jb, c]

            with tc.tile_pool(
                name="proj_ps", bufs=2, space=bass.MemorySpace.PSUM
            ) as pps:
                # k = wk h + bk : 4 psum tiles of [128, 1024]
                for t4 in range(4):
                    kps = pps.tile([C, I], F32, tag="kq")
                    for ih in range(2):
                        s = t4 * 1024 + ih * 512
                        nc.tensor.matmul(
                            kps[:, ih * 512:(ih + 1) * 512],
                            wk_r[:], h_t[:, s:s + 512],
                            start=True, stop=True,
                        )
                    nc.vector.tensor_scalar(
                        out=k_t[:, t4 * 1024:(t4 + 1) * 1024], in0=kps[:],
                        scalar1=bk_t[:], scalar2=None, op0=mybir.AluOpType.add,
                    )
                # q = wq cn + bq
                qps = pps.tile([C, I], F32, tag="kq")
                for ih in range(2):
                    nc.tensor.matmul(
                        qps[:, ih * 512:(ih + 1) * 512],
                        wq_r[:], cn_t[:, ih * 512:(ih + 1) * 512],
                        start=True, stop=True,
                    )
                nc.vector.tensor_scalar(
                    out=q_t[:], in0=qps[:], scalar1=bq_t[:], scalar2=None,
                    op0=mybir.AluOpType.add,
                )
                # v^T blocks: vt[j, c] = sum_cin h[cin, j] wv[c, cin]
                # (bias bv folded into btp on the host)
                for jb in range(JB):
                    vps = pps.tile([C, C], F32, tag="vt")
                    nc.tensor.matmul(
                        vps[:], h_t[:, jb * 128:(jb + 1) * 128], wv_r[:],
                        start=True, stop=True,
                    )
                    nc.vector.tensor_copy(vt_t[:, jb, :], vps[:])

            if DEBUG_OUTS:
                nc.sync.dma_start(dbg["dbg_k"][:], k_t[:].bitcast(F32))
                nc.sync.dma_start(dbg["dbg_q"][:], q_t[:].bitcast(F32))
                nc.sync.dma_start(
                    dbg["dbg_vt"][:],
                    vt_t[:].bitcast(F32).rearrange("p jb c -> p (jb c)"),
                )

            # ---- attention ----
            o_sb = per.tile([C, I], MM_DT, tag="osb")
            rb_sb = per.tile([C, I], F32, tag="rbsb")
            with tc.tile_pool(
                name="acc_ps", bufs=1, space=bass.MemorySpace.PSUM
            ) as acc:
                o_ps = acc.tile([C, I], F32, tag="o")
                d_ps = acc.tile([1, I], F32, tag="d")
                with tc.tile_pool(
                    name="st_ps", bufs=2, space=bass.MemorySpace.PSUM
                ) as stp:
                    for jb in range(JB):
                        st = stp.tile([C, I], F32, tag="st")
                        for ih in range(2):
                            nc.tensor.matmul(
                                st[:, ih * 512:(ih + 1) * 512],
                                k_t[:, jb * 128:(jb + 1) * 128],
                                q_t[:, ih * 512:(ih + 1) * 512],
                                start=True, stop=True,
                            )
                        p_t = pp.tile([C, I], MM_DT, tag="p")
                        nc.scalar.activation(
                            out=p_t[:], in_=st[:],
                            func=mybir.ActivationFunctionType.Exp,
                            bias=zero128_t[:], scale=float(SCALE),
                        )
                        first, last = jb == 0, jb == JB - 1
                        for ih in range(2):
                            sl = slice(ih * 512, (ih + 1) * 512)
                            nc.tensor.matmul(
                                o_ps[:, sl], vt_t[:, jb, :], p_t[:, sl],
                                start=first, stop=last,
                            )
                            nc.tensor.matmul(
                                d_ps[:, sl], ones_r[:], p_t[:, sl],
                                start=first, stop=last,
                            )

                # softmax denominator -> reciprocal via exp(-ln(d))
                lnd = sm.tile([1, I], F32, tag="lnd")
                nc.scalar.activation(
                    out=lnd[:], in_=d_ps[:],
                    func=mybir.ActivationFunctionType.Ln, bias=zero1_t[:], scale=1.0,
                )
                if DEBUG_OUTS:
                    dcp = sm.tile([1, I], F32, tag="dcp")
                    nc.vector.tensor_copy(dcp[:], d_ps[:])
                    nc.sync.dma_start(dbg["dbg_d"][:], dcp[:])
                rsb = sm.tile([1, I], F32, tag="rsb")
                nc.scalar.activation(
                    out=rsb[:], in_=lnd[:],
                    func=mybir.ActivationFunctionType.Exp, bias=zero1_t[:], scale=-1.0,
                )

                with tc.tile_pool(
                    name="tail_ps", bufs=1, space=bass.MemorySpace.PSUM
                ) as tlp:
                    rb_ps = tlp.tile([C, I], F32, tag="rb")
                    for ih in range(2):
                        sl = slice(ih * 512, (ih + 1) * 512)
                        # exact fp32 broadcast along partitions (K=1 matmul)
                        nc.tensor.matmul(
                            rb_ps[:, sl], ones1_t[:], rsb[:, sl],
                            start=True, stop=True,
                        )
                    nc.vector.tensor_copy(rb_sb[:], rb_ps[:])
                # normalize attention output while copying out of PSUM
                nc.vector.tensor_tensor(
                    o_sb[:], o_ps[:], rb_sb[:], mybir.AluOpType.mult
                )
                if DEBUG_OUTS:
                    nc.sync.dma_start(dbg["dbg_o"][:], o_sb[:].bitcast(F32))

            # ---- projection + residual ----
            f_t = per.tile([C, I], F32, tag="f")
            with tc.tile_pool(
                name="z_ps", bufs=1, space=bass.MemorySpace.PSUM
            ) as zp:
                z_ps = zp.tile([C, I], F32, tag="z")
                for ih in range(2):
                    sl = slice(ih * 512, (ih + 1) * 512)
                    nc.tensor.matmul(
                        z_ps[:, sl], wp_r[:], o_sb[:, sl],
                        start=True, stop=True,
                    )
                nc.vector.tensor_scalar(
                    out=f_t[:], in0=z_ps[:], scalar1=btp_t[:], scalar2=None,
                    op0=mybir.AluOpType.add,
                )
            nc.vector.tensor_add(f_t[:], f_t[:], xsl_t[:])
            nc.sync.dma_start(y[:], f_t[:])

    cap_sync_waits(nc)
    return nc


_PROGRAM = None


def _get_program():
    global _PROGRAM
    if _PROGRAM is None:
        _PROGRAM = build_program()
    return _PROGRAM


def _prep_in_maps(x, c, g1, b1, g2, b2, wq, bq, wk, bk, wv, bv, wp, bp):
    f = np.float32
    col = lambda v: np.ascontiguousarray(np.asarray(v, f).reshape(C, 1))
    ch = np.arange(C) // 32
    gavg = np.zeros((C, NG), f)
    gavg[np.arange(C), ch] = 1.0 / 32.0
    bc4 = np.zeros((NG, C), f)
    bc4[ch, np.arange(C)] = 1.0
    common = {
        "wqT": np.ascontiguousarray(np.asarray(wq, f).T),
        "wkT": np.ascontiguousarray(np.asarray(wk, f).T),
        "wvT": np.ascontiguousarray(np.asarray(wv, f).T),
        "wpT": np.ascontiguousarray(np.asarray(wp, f).T),
        "bq": col(bq),
        "bk": col(bk),
        "btp": col(np.asarray(wp, f) @ np.asarray(bv, f) + np.asarray(bp, f)),
        "g1": col(g1), "b1": col(b1), "g2": col(g2), "b2": col(b2),
        "gavg": gavg, "bc4": bc4,
        "ones": np.ones((C, 1), f), "ones1": np.ones((1, C), f),
    }
    xf = np.asarray(x, f).reshape(2, C, N)
    cf = np.asarray(c, f).reshape(2, C, N)
    in_maps = []
    for m in range(N_CORES):
        b, quarter = m // 4, m % 4
        i0 = quarter * I
        in_maps.append({
            "xb": np.ascontiguousarray(xf[b]),
            "cb": np.ascontiguousarray(cf[b]),
            "x_sl": np.ascontiguousarray(xf[b][:, i0:i0 + I]),
            "c_sl": np.ascontiguousarray(cf[b][:, i0:i0 + I]),
            **common,
        })
    return in_maps


def run_spmd(inputs, trace=False, **kw):
    nc = _get_program()
    in_maps = _prep_in_maps(**inputs)
    return run_bass_kernel_spmd(nc, in_maps, list(range(N_CORES)), trace=trace, **kw)


def kernel(**inputs) -> np.ndarray:
    res = run_spmd(inputs, trace=False)
    out = np.empty((2, C, N), np.float32)
    for m in range(N_CORES):
        b, quarter = m // 4, m % 4
        out[b][:, quarter * I:(quarter + 1) * I] = res.results[m]["y"]
    return out.reshape(2, C, 16, 16, 16)
